# revision 44
# baseline (speedup 1.0000x reference)
"""Nystrom attention Trainium2 kernel (v2).

Sharding: 8 cores = 4 batches x 2 head-groups (4 heads each). Each core
computes its (batch, head-group) slice end-to-end including its share of the
output projection; the host sums the two partial projections per batch and
adds bo.

Key structure (single fused pipeline per core):
  - q/k projections run in fp8e4 with DoubleRow perf mode (W pre-scaled by
    32 on the host; all q/k-derived tensors carry the 32x factor, removed in
    the exp activations via scale=1/1024).
  - v projection stays bf16 (fp8 v fails the accuracy budget); the v bias is
    folded into the normalized t1 (kernel_3 rows sum to 1).
  - landmark means are computed from host-precomputed per-segment x means
    (linearity: mean(xW) = mean(x)W), so landmarks + kernel_2 + the
    Newton-Schulz inverse all run concurrently with phase A.
  - Newton-Schulz runs on 2-head block-diagonal [128,128] tiles, bf16 for
    iters 0-4 and f32 for the final iteration.
  - Phase A also computes ps1 -> e1 (kernel_1 numerator), stored in SBUF.
  - Phase B per chunk: prb (denominator via ones-block matmul), pht (t2
    apply), reciprocal+multiply normalize, psout, bf16 output DMA.
"""

from contextlib import ExitStack

import numpy as np
import ml_dtypes

import concourse.bass as bass
import concourse.tile as tile
from concourse import bacc, mybir
from concourse.bass_utils import run_bass_kernel_spmd

BF16 = mybir.dt.bfloat16
F32 = mybir.dt.float32
FP8 = mybir.dt.float8e4
AF = mybir.ActivationFunctionType
AX = mybir.AxisListType
OP = mybir.AluOpType
DR = mybir.MatmulPerfMode.DoubleRow

S = 8192        # sequence length
E = 512         # embedding dim
D = 64          # head dim
L = 64          # landmarks
N_ITER = 6
SCALE = 1.0 / np.sqrt(np.sqrt(D))
W8 = 32.0                    # fp8 / bf16 weight pre-scale for q,k
EXPSC = 1.0 / (W8 * W8)      # removes the 32x q * 32x k factor inside exp

_CACHED_NC = None
DEBUG_TAPS = False


def _build():
    nc = bacc.Bacc("TRN2", target_bir_lowering=False, debug=False, num_devices=8)

    x8_d = nc.dram_tensor("x8", [E, S], FP8, kind="ExternalInput").ap()
    xb_d = nc.dram_tensor("xb", [E, S], BF16, kind="ExternalInput").ap()
    xsegT_d = nc.dram_tensor("xsegT", [E, L], BF16, kind="ExternalInput").ap()
    wqk8_d = nc.dram_tensor("wqk8", [E, 512], FP8, kind="ExternalInput").ap()
    wqkb_d = nc.dram_tensor("wqkb", [E, 512], BF16, kind="ExternalInput").ap()
    wv_d = nc.dram_tensor("wv", [E, 256], BF16, kind="ExternalInput").ap()
    wo_d = nc.dram_tensor("wo", [256, E], BF16, kind="ExternalInput").ap()
    bqk_d = nc.dram_tensor("bqk", [512], F32, kind="ExternalInput").ap()
    idf32_d = nc.dram_tensor("idf32", [128, 128], F32, kind="ExternalInput").ap()
    nsc_d = nc.dram_tensor("nsc", [128, 384], F32, kind="ExternalInput").ap()
    onesr_d = nc.dram_tensor("onesr", [1, 128], F32, kind="ExternalInput").ap()
    blk1_d = nc.dram_tensor("blk1", [128, 128], BF16, kind="ExternalInput").ap()
    out_d = nc.dram_tensor("out", [S, E], BF16, kind="ExternalOutput").ap()
    taps = None
    if DEBUG_TAPS:
        taps = {
            "dbg_landqk": nc.dram_tensor("dbg_landqk", [128, 4, L], BF16,
                                         kind="ExternalOutput").ap(),
            "dbg_qT": nc.dram_tensor("dbg_qT", [128, 2, S], BF16,
                                     kind="ExternalOutput").ap(),
            "dbg_kT": nc.dram_tensor("dbg_kT", [128, 2, S], BF16,
                                     kind="ExternalOutput").ap(),
            "dbg_vsb": nc.dram_tensor("dbg_vsb", [128, 64, 4, 65], BF16,
                                      kind="ExternalOutput").ap(),
            "dbg_e1": nc.dram_tensor("dbg_e1", [128, 2, S], BF16,
                                     kind="ExternalOutput").ap(),
            "dbg_t2": nc.dram_tensor("dbg_t2", [128, 2, 128], BF16,
                                     kind="ExternalOutput").ap(),
        }

    with tile.TileContext(nc) as tc:
        _emit(nc, tc, x8_d, xb_d, xsegT_d, wqk8_d, wqkb_d, wv_d, wo_d, bqk_d,
              idf32_d, nsc_d, onesr_d, blk1_d, out_d, taps)
    nc.compile()
    return nc


def _emit(nc, tc, x8_d, xb_d, xsegT_d, wqk8_d, wqkb_d, wv_d, wo_d, bqk_d,
          idf32_d, nsc_d, onesr_d, blk1_d, out_d, taps=None):
    with (
        tc.tile_pool(name="const", bufs=1) as const,
        tc.tile_pool(name="big", bufs=1) as big,
        tc.tile_pool(name="small", bufs=2) as small,
    ):
        # ---- constants / weights into SBUF ----
        wqkb_sb = const.tile([128, 4, 512], BF16, tag="wqkb")
        nc.sync.dma_start(wqkb_sb[:], wqkb_d.rearrange("(ko p) m -> p ko m", p=128))
        xsegT_sb = const.tile([128, 4, L], BF16, tag="xsegT")
        nc.sync.dma_start(xsegT_sb[:], xsegT_d.rearrange("(ko p) l -> p ko l", p=128))
        bqk_sb = const.tile([128, 4], F32, tag="bqk")
        nc.sync.dma_start(bqk_sb[:], bqk_d.rearrange("(t p) -> p t", p=128))
        wqk8_sb = const.tile([128, 4, 512], FP8, tag="wqk8")
        nc.sync.dma_start(wqk8_sb[:], wqk8_d.rearrange("(ko p) m -> p ko m", p=128))
        wv_sb = const.tile([128, 4, 256], BF16, tag="wv")
        nc.sync.dma_start(wv_sb[:], wv_d.rearrange("(ko p) m -> p ko m", p=128))
        # non-critical consts DMA'd after the first x chunks (see below)
        wo_sb = const.tile([128, 2, 512], BF16, tag="wo")
        idf32_sb = const.tile([128, 128], F32, tag="idf32")
        nsc_sb = const.tile([128, 384], F32, tag="nsc")
        onesr_sb = const.tile([1, 128], F32, tag="onesr")
        blk1_sb = const.tile([128, 128], BF16, tag="blk1")

        # ---- persistent activations ----
        qT = big.tile([128, 2, S], BF16, tag="qT")       # 32x q, [(2h d) | s]
        kT = big.tile([128, 2, S], BF16, tag="kT")
        vsb = big.tile([128, 64, 4, 65], BF16, tag="v")  # [s | chunk, head, d+1]
        e1sb = big.tile([128, 2, S], BF16, tag="e1")     # exp(kernel_1 logits)
        nc.vector.memset(vsb[:, :, :, 64:65], 1.0)

        landqk = const.tile([128, 4, L], BF16, tag="landqk")  # 32x landmarks
        qblk = []
        kblk = []
        t2blk = []
        for hp in range(2):
            qb = const.tile([128, 128], BF16, tag=f"qblk{hp}")
            kb = const.tile([128, 128], BF16, tag=f"kblk{hp}")
            tb = const.tile([128, 128], BF16, tag=f"t2blk{hp}")
            for b_ in (qb, kb, tb):
                nc.vector.memset(b_[:], 0.0)
            qblk.append(qb)
            kblk.append(kb)
            t2blk.append(tb)

        x8_t = x8_d.rearrange("(ko p) s -> p ko s", p=128)
        xb_t = xb_d.rearrange("(ko p) s -> p ko s", p=128)

        HSL = [slice(0, 64), slice(64, 128)]

        # =================== Phase A + NS ===================
        with (
            tc.tile_pool(name="ps_t1", bufs=1, space="PSUM") as ps_t1,
            tc.tile_pool(name="ps_ns", bufs=1, space="PSUM") as ps_ns,
            tc.tile_pool(name="nsp", bufs=2) as nsp,
        ):
            _stk = ExitStack()
            xpool = _stk.enter_context(tc.tile_pool(name="xt", bufs=3))
            ps_qk = _stk.enter_context(
                tc.tile_pool(name="ps_qk", bufs=2, space="PSUM"))
            ps_v = _stk.enter_context(
                tc.tile_pool(name="ps_v", bufs=2, space="PSUM"))
            ps_s3 = _stk.enter_context(
                tc.tile_pool(name="ps_s3", bufs=2, space="PSUM"))
            e3p = _stk.enter_context(tc.tile_pool(name="e3p", bufs=3))
            # prefetch first x chunks ahead of the non-critical consts
            xt8s = {}
            xtbs = {}

            def fetch(c):
                slc = bass.ts(c, 512)
                a = xpool.tile([128, 4, 512], FP8, tag="xt8", name=f"xt8_{c}")
                nc.sync.dma_start(a[:], x8_t[:, :, slc])
                bb = xpool.tile([128, 4, 512], BF16, tag="xtb", name=f"xtb_{c}")
                nc.sync.dma_start(bb[:], xb_t[:, :, slc])
                xt8s[c] = a
                xtbs[c] = bb

            fetch(0)
            fetch(1)
            nc.sync.dma_start(idf32_sb[:], idf32_d[:])
            nc.sync.dma_start(nsc_sb[:], nsc_d[:])
            nc.sync.dma_start(onesr_sb[:], onesr_d[:])
            nc.sync.dma_start(wo_sb[:], wo_d.rearrange("(j p) m -> p j m", p=128))
            nc.sync.dma_start(blk1_sb[:], blk1_d[:])

            # ---- landmarks from host-precomputed segment means ----
            ps_land = ps_ns.tile([128, 4, L], F32, tag="ns", name="ps_land")
            for t in range(4):
                for ko in range(4):
                    nc.tensor.matmul(
                        ps_land[:, t, :], lhsT=wqkb_sb[:, ko, bass.ts(t, 128)],
                        rhs=xsegT_sb[:, ko, :], start=(ko == 0), stop=(ko == 3))
            for t in range(4):
                nc.scalar.activation(landqk[:, t, :], ps_land[:, t, :],
                                     AF.Identity, bias=bqk_sb[:, t:t + 1])
            for hp in range(2):
                for h2 in range(2):
                    psl = HSL[h2]
                    nc.vector.tensor_copy(qblk[hp][psl, psl], landqk[psl, hp, :])
                    nc.vector.tensor_copy(kblk[hp][psl, psl],
                                          landqk[psl, 2 + hp, :])

            # ---- kernel_2 (block-diag, both heads of hp at once) ----
            K2blk = []
            for hp in range(2):
                ps2 = ps_ns.tile([128, 128], F32, tag="ns", name=f"ps2_{hp}")
                nc.tensor.matmul(ps2[:], lhsT=qblk[hp], rhs=kblk[hp],
                                 start=True, stop=True)
                k2e = nsp.tile([128, 128], F32, tag=f"k2e{hp}", name=f"k2e{hp}")
                nc.vector.memset(k2e[:], 0.0)
                rs = small.tile([128, 1], F32, tag="k2rs")
                for h2 in range(2):
                    psl = HSL[h2]
                    nc.scalar.activation(k2e[psl, psl], ps2[psl, psl], AF.Exp,
                                         scale=EXPSC, accum_out=rs[psl, :])
                ri = small.tile([128, 1], F32, tag="k2ri")
                nc.vector.reciprocal(ri[:], rs[:])
                K2 = nsp.tile([128, 128], F32, tag=f"K2_{hp}", name=f"K2_{hp}")
                nc.vector.tensor_scalar_mul(K2[:], k2e[:], ri[:])
                K2blk.append(K2)

            # ---- Newton-Schulz init (block-diag pairs) ----
            K2Tf = []
            K2Tb = []
            Vb = []
            Wb = []
            for hp in range(2):
                pk = ps_ns.tile([128, 128], F32, tag="ns", name=f"pk{hp}")
                nc.tensor.transpose(pk[:], K2blk[hp][:], idf32_sb[:])
                k2tf = nsp.tile([128, 128], F32, tag=f"K2Tf{hp}", name=f"K2Tf{hp}")
                nc.vector.tensor_copy(k2tf[:], pk[:])
                k2tb = nsp.tile([128, 128], BF16, tag=f"K2Tb{hp}", name=f"K2Tb{hp}")
                nc.scalar.activation(k2tb[:], pk[:], AF.Identity)
                K2Tf.append(k2tf)
                K2Tb.append(k2tb)

                cs = nsp.tile([128, 1], F32, tag=f"cs{hp}", name=f"cs{hp}")
                nc.vector.reduce_sum(cs[:], k2tf[:], axis=AX.X)
                pc = ps_ns.tile([1, 128], F32, tag="ns", name=f"pc{hp}")
                nc.tensor.transpose(pc[0:1, :], cs[:], idf32_sb[:])
                # one init scale per head pair (NS convergence is insensitive
                # to a constant factor in the init scale)
                mx = nsp.tile([1, 1], F32, tag=f"mx{hp}", name=f"mx{hp}")
                nc.vector.reduce_max(mx[:], pc[0:1, :], axis=AX.X)
                pb = ps_ns.tile([128, 1], F32, tag="ns", name=f"pb{hp}")
                nc.tensor.matmul(pb[:, 0:1], lhsT=onesr_sb[0:1, :],
                                 rhs=mx[:], start=True, stop=True)
                mxi = nsp.tile([128, 1], F32, tag=f"mxi{hp}", name=f"mxi{hp}")
                nc.vector.reciprocal(mxi[:], pb[:])
                v0 = nsp.tile([128, 128], BF16, tag=f"V{hp}", name=f"V{hp}")
                nc.vector.tensor_scalar_mul(v0[:], k2tf[:], mxi[:])
                w0 = nsp.tile([128, 128], BF16, tag=f"W{hp}", name=f"W{hp}")
                nc.vector.tensor_scalar_mul(w0[:], K2blk[hp][:], mxi[:])
                Vb.append(v0)
                Wb.append(w0)

            # ---- Newton-Schulz iterations ----
            # iters 0..4 in bf16, final iteration in f32
            Vcur, Wcur = Vb, Wb
            for it in range(N_ITER):
                last = it == N_ITER - 1
                dt_i = F32 if last else BF16
                K2T = K2Tf if last else K2Tb
                if last:
                    Vf, Wf = [], []
                    for hp in range(2):
                        vf = nsp.tile([128, 128], F32, tag=f"Vf{hp}",
                                      name=f"Vf{hp}")
                        nc.gpsimd.tensor_copy(vf[:], Vcur[hp][:])
                        wf = nsp.tile([128, 128], F32, tag=f"Wf{hp}",
                                      name=f"Wf{hp}")
                        nc.gpsimd.tensor_copy(wf[:], Wcur[hp][:])
                        Vf.append(vf)
                        Wf.append(wf)
                    Vcur, Wcur = Vf, Wf
                Vnew, Wnew = [], []
                for hp in range(2):
                    pkv = ps_ns.tile([128, 128], F32, tag="ns",
                                     name=f"pkv{it}{hp}")
                    nc.tensor.matmul(pkv[:], lhsT=K2T[hp][:], rhs=Vcur[hp][:],
                                     start=True, stop=True)
                    T1 = nsp.tile([128, 128], dt_i, tag=f"T1{hp}",
                                  name=f"T1_{it}{hp}")
                    nc.vector.tensor_tensor(T1[:], nsc_sb[:, 0:128], pkv[:],
                                            op=OP.subtract)
                    pvt = ps_ns.tile([128, 128], F32, tag="ns",
                                     name=f"pvt{it}{hp}")
                    nc.tensor.matmul(pvt[:], lhsT=Vcur[hp][:], rhs=K2T[hp][:],
                                     start=True, stop=True)
                    KVT = nsp.tile([128, 128], dt_i, tag=f"KVT{hp}",
                                   name=f"KVT_{it}{hp}")
                    nc.scalar.activation(KVT[:], pvt[:], AF.Identity)
                    p3 = ps_ns.tile([128, 128], F32, tag="ns",
                                    name=f"p3{it}{hp}")
                    nc.tensor.matmul(p3[:], lhsT=KVT[:], rhs=T1[:],
                                     start=True, stop=True)
                    T2 = nsp.tile([128, 128], dt_i, tag=f"T2{hp}",
                                  name=f"T2_{it}{hp}")
                    nc.vector.tensor_tensor(T2[:], nsc_sb[:, 128:256], p3[:],
                                            op=OP.subtract)
                    p4 = ps_ns.tile([128, 128], F32, tag="ns",
                                    name=f"p4{it}{hp}")
                    nc.tensor.matmul(p4[:], lhsT=KVT[:], rhs=T2[:],
                                     start=True, stop=True)
                    T3 = nsp.tile([128, 128], dt_i, tag=f"T3{hp}",
                                  name=f"T3_{it}{hp}")
                    nc.vector.scalar_tensor_tensor(
                        T3[:], p4[:], -0.25, nsc_sb[:, 256:384],
                        op0=OP.mult, op1=OP.add)
                    p5 = ps_ns.tile([128, 128], F32, tag="ns",
                                    name=f"p5{it}{hp}")
                    nc.tensor.matmul(p5[:], lhsT=Wcur[hp][:], rhs=T3[:],
                                     start=True, stop=True)
                    p6 = ps_ns.tile([128, 128], F32, tag="ns",
                                    name=f"p6{it}{hp}")
                    nc.tensor.matmul(p6[:], lhsT=T3[:], rhs=Wcur[hp][:],
                                     start=True, stop=True)
                    vn = nsp.tile([128, 128], dt_i, tag=f"V{hp}" if not last
                                  else f"Vf{hp}", name=f"Vn_{it}{hp}")
                    nc.scalar.activation(vn[:], p5[:], AF.Identity)
                    wn = nsp.tile([128, 128], dt_i, tag=f"W{hp}" if not last
                                  else f"Wf{hp}", name=f"Wn_{it}{hp}")
                    nc.scalar.activation(wn[:], p6[:], AF.Identity)
                    Vnew.append(vn)
                    Wnew.append(wn)
                Vcur, Wcur = Vnew, Wnew

            # ---- main chunk loop: qkv projections, e3, t1 accumulation ----
            t1all = ps_t1.tile([65, 256], F32, tag="t1", name="t1ps")
            t1ps = [t1all[:, 0:128], t1all[:, 128:256]]
            # all four (hp, h2) groups share one PSUM bank: a start=True would
            # mark the whole 2KB zero-region and wipe sibling partials, so
            # zero once and accumulate with start=False
            nc.vector.memset(t1all[:], 0.0)
            for c in range(16):
                sl = bass.ts(c, 512)
                if c + 2 < 16:
                    fetch(c + 2)
                xt8 = xt8s.pop(c)
                xtb = xtbs.pop(c)
                for t in range(4):  # q01 q23 k01 k23
                    ps = ps_qk.tile([128, 512], F32, tag="psqk")
                    for kp in range(2):
                        nc.tensor.matmul(
                            ps[:],
                            lhsT=wqk8_sb[:, 2 * kp:2 * kp + 2, bass.ts(t, 128)],
                            rhs=xt8[:, 2 * kp:2 * kp + 2, :],
                            start=(kp == 0), stop=(kp == 1), perf_mode=DR)
                    dst = qT if t < 2 else kT
                    hp = t % 2
                    nc.vector.tensor_scalar_add(dst[:, hp, sl], ps[:],
                                                bqk_sb[:, t:t + 1])
                for sp in range(2):  # pairs of 128-row s chunks
                    psv = ps_v.tile([128, 2, 256], F32, tag="psv")
                    for j in range(2):
                        s4 = sp * 2 + j
                        for ko in range(4):
                            nc.tensor.matmul(
                                psv[:, j, :], lhsT=xtb[:, ko, bass.ts(s4, 128)],
                                rhs=wv_sb[:, ko, :], start=(ko == 0),
                                stop=(ko == 3))
                    if (c * 2 + sp) % 3 == 0:
                        nc.vector.tensor_copy(
                            vsb[:, c * 4 + sp * 2:c * 4 + sp * 2 + 2, :, 0:64],
                            psv[:].rearrange("p j (h d) -> p j h d", d=64))
                    else:
                        nc.scalar.activation(
                            vsb[:, c * 4 + sp * 2:c * 4 + sp * 2 + 2, :, 0:64],
                            psv[:].rearrange("p j (h d) -> p j h d", d=64),
                            AF.Identity)
                for hp in range(2):
                    ps3 = ps_s3.tile([128, 512], F32, tag="ps3")
                    for i in range(4):
                        nc.tensor.matmul(
                            ps3[:, bass.ts(i, 128)],
                            lhsT=kT[:, hp, bass.ts(c * 4 + i, 128)],
                            rhs=qblk[hp], start=True, stop=True)
                    e3 = e3p.tile([128, 512], BF16, tag="e3")
                    nc.scalar.activation(e3[:], ps3[:], AF.Exp, scale=EXPSC)
                    for i in range(4):
                        cc = c * 4 + i
                        for h2 in range(2):
                            h = hp * 2 + h2
                            nc.tensor.matmul(
                                t1ps[hp][:, bass.ts(h2, 64)],
                                lhsT=vsb[:, cc, h, :],
                                rhs=e3[:, i * 128 + h2 * 64:
                                       i * 128 + h2 * 64 + 64],
                                start=False, stop=(cc == 63),
                                skip_group_check=True)
                    ps1 = ps_s3.tile([128, 512], F32, tag="ps3", name=f"ps1_{c}{hp}")
                    nc.tensor.matmul(ps1[:], lhsT=kblk[hp], rhs=qT[:, hp, sl],
                                     start=True, stop=True)
                    nc.scalar.activation(e1sb[:, hp, sl], ps1[:], AF.Exp,
                                         scale=EXPSC)

            _stk.close()  # free chunk-loop pools (SBUF + 6 PSUM banks)

            # ---- NS tail: normalize t1, add v bias, t2 = V2 @ t1n ----
            with tc.tile_pool(name="ps_tail", bufs=4, space="PSUM") as ps_tail:
                for hp in range(2):
                    t1u = nsp.tile([65, 128], F32, tag=f"t1u{hp}",
                                   name=f"t1u{hp}")
                    nc.vector.tensor_copy(t1u[:], t1ps[hp][:])
                    ptt = ps_tail.tile([128, 65], F32, tag="tl",
                                       name=f"ptt{hp}")
                    nc.tensor.transpose(ptt[:], t1u[:], idf32_sb[0:65, 0:65])
                    d3i = nsp.tile([128, 1], F32, tag=f"d3i{hp}",
                                   name=f"d3i{hp}")
                    nc.vector.reciprocal(d3i[:], ptt[:, 64:65])
                    # ptt rows are already (h2, l); t1n blocks are [l, d]
                    t1n = nsp.tile([128, 64], F32, tag=f"t1n{hp}",
                                   name=f"t1n{hp}")
                    nc.vector.tensor_scalar_mul(t1n[:], ptt[:, 0:64], d3i[:])
                    t1nblk = nsp.tile([128, 128], F32, tag=f"t1nblk{hp}",
                                      name=f"t1nblk{hp}")
                    nc.vector.memset(t1nblk[:], 0.0)
                    for h2 in range(2):
                        psl = HSL[h2]
                        nc.vector.tensor_copy(t1nblk[psl, psl], t1n[psl, :])
                    pt2 = ps_tail.tile([128, 128], F32, tag="tl",
                                       name=f"pt2{hp}")
                    nc.tensor.matmul(pt2[:], lhsT=Wcur[hp][:], rhs=t1nblk[:],
                                     start=True, stop=True)
                    nc.vector.tensor_copy(t2blk[hp][:], pt2[:])

        # ======= Phase B: kernel_1, normalize, output projection =======
        with (
            tc.tile_pool(name="ps_rb", bufs=2, space="PSUM") as ps_rb,
            tc.tile_pool(name="ps_ht", bufs=2, space="PSUM") as ps_ht,
            tc.tile_pool(name="ps_out", bufs=4, space="PSUM") as ps_out,
            tc.tile_pool(name="hcp", bufs=4) as hcp,
        ):
            for c in range(16):
                sl = bass.ts(c, 512)
                hcts = []
                for hp in range(2):
                    prb = ps_rb.tile([128, 512], F32, tag="prb")
                    nc.tensor.matmul(prb[:], lhsT=blk1_sb[:],
                                     rhs=e1sb[:, hp, sl], start=True, stop=True)
                    pht = ps_ht.tile([128, 512], F32, tag="pht")
                    nc.tensor.matmul(pht[:], lhsT=t2blk[hp],
                                     rhs=e1sb[:, hp, sl], start=True, stop=True)
                    rbs = hcp.tile([128, 512], F32, tag="rbs")
                    nc.vector.reciprocal(rbs[:], prb[:])
                    hct = hcp.tile([128, 512], BF16, tag="hct")
                    nc.vector.tensor_tensor(hct[:], pht[:], rbs[:],
                                            op=OP.mult)
                    hcts.append(hct)
                for s4 in range(4):
                    c128 = c * 4 + s4
                    pso = ps_out.tile([128, 512], F32, tag="psout")
                    for hp in range(2):
                        nc.tensor.matmul(pso[:],
                                         lhsT=hcts[hp][:, bass.ts(s4, 128)],
                                         rhs=wo_sb[:, hp, :],
                                         start=(hp == 0), stop=(hp == 1))
                    osb = hcp.tile([128, 512], BF16, tag="osb")
                    nc.scalar.activation(osb[:], pso[:], AF.Identity)
                    nc.sync.dma_start(out_d[bass.ts(c128, 128), :], osb[:])
        if taps is not None:
            nc.sync.dma_start(taps["dbg_landqk"][:], landqk[:])
            nc.sync.dma_start(taps["dbg_qT"][:], qT[:])
            nc.sync.dma_start(taps["dbg_kT"][:], kT[:])
            nc.sync.dma_start(taps["dbg_vsb"][:], vsb[:])
            nc.sync.dma_start(taps["dbg_e1"][:], e1sb[:])
            for hp in range(2):
                nc.sync.dma_start(taps["dbg_t2"][:, hp, :], t2blk[hp][:])


def _prep_inputs(x, Wq, bq, Wk, bk, Wv, bv, Wo, bo):
    bf = ml_dtypes.bfloat16
    f8 = ml_dtypes.float8_e4m3
    x = np.asarray(x, dtype=np.float32)
    Wq = np.asarray(Wq, dtype=np.float32)
    Wk = np.asarray(Wk, dtype=np.float32)
    Wv = np.asarray(Wv, dtype=np.float32)
    Wo = np.asarray(Wo, dtype=np.float32)
    bq = np.asarray(bq, dtype=np.float32)
    bk = np.asarray(bk, dtype=np.float32)
    bv = np.asarray(bv, dtype=np.float32)

    i64 = np.eye(64, dtype=np.float32)
    i128 = np.eye(128, dtype=np.float32)
    consts = {
        "idf32": i128,
        "nsc": np.ascontiguousarray(np.concatenate(
            [7.0 * i128, 15.0 * i128, 3.25 * i128], axis=1)),
        "onesr": np.ones((1, 128), dtype=np.float32),
        "blk1": np.ascontiguousarray(
            np.kron(np.eye(2), np.ones((64, 64))).astype(bf)),
    }
    in_maps = []
    for core in range(8):
        b, g = core // 2, core % 2
        hsl = slice(g * 256, (g + 1) * 256)
        xT = np.ascontiguousarray(x[b].T)
        xseg = x[b].reshape(L, S // L, E).mean(axis=1)  # [L, E]
        wqk = np.ascontiguousarray(
            np.concatenate([Wq[:, hsl], Wk[:, hsl]], axis=1) * (SCALE * W8))
        in_maps.append({
            "x8": xT.astype(f8),
            "xb": xT.astype(bf),
            "xsegT": np.ascontiguousarray(xseg.T).astype(bf),
            "wqk8": wqk.astype(f8),
            "wqkb": wqk.astype(bf),
            "wv": np.ascontiguousarray(Wv[:, hsl]).astype(bf),
            "wo": np.ascontiguousarray(Wo[hsl, :]).astype(bf),
            "bqk": np.ascontiguousarray(
                np.concatenate([bq[hsl], bk[hsl]]) * (SCALE * W8)
            ).astype(np.float32),
            **consts,
        })
    return in_maps


def run_on_device(in_maps, **kwargs):
    global _CACHED_NC
    if _CACHED_NC is None:
        _CACHED_NC = _build()
    return run_bass_kernel_spmd(_CACHED_NC, in_maps, core_ids=list(range(8)),
                                **kwargs)


def kernel(x, Wq, bq, Wk, bk, Wv, bv, Wo, bo):
    in_maps = _prep_inputs(x, Wq, bq, Wk, bk, Wv, bv, Wo, bo)
    res = run_on_device(in_maps)
    bo = np.asarray(bo, dtype=np.float32) + (
        np.asarray(bv, dtype=np.float32) @ np.asarray(Wo, dtype=np.float32))
    out = np.empty((4, S, E), dtype=np.float32)
    for b in range(4):
        out[b] = (res.results[2 * b]["out"].astype(np.float32)
                  + res.results[2 * b + 1]["out"].astype(np.float32) + bo)
    return out


# revision 51
# speedup vs baseline: 1.0923x; 1.0923x over previous
"""Nystrom attention Trainium2 kernel (v2).

Sharding: 8 cores = 4 batches x 2 head-groups (4 heads each). Each core
computes its (batch, head-group) slice end-to-end including its share of the
output projection; the host sums the two partial projections per batch and
adds bo.

Key structure (single fused pipeline per core):
  - q/k projections run in fp8e4 with DoubleRow perf mode (W pre-scaled by
    32 on the host; all q/k-derived tensors carry the 32x factor, removed in
    the exp activations via scale=1/1024).
  - v projection stays bf16 (fp8 v fails the accuracy budget); the v bias is
    folded into the normalized t1 (kernel_3 rows sum to 1).
  - landmark means are computed from host-precomputed per-segment x means
    (linearity: mean(xW) = mean(x)W), so landmarks + kernel_2 + the
    Newton-Schulz inverse all run concurrently with phase A.
  - Newton-Schulz runs on 2-head block-diagonal [128,128] tiles, bf16 for
    iters 0-4 and f32 for the final iteration.
  - Phase A also computes ps1 -> e1 (kernel_1 numerator), stored in SBUF.
  - Phase B per chunk: prb (denominator via ones-block matmul), pht (t2
    apply), reciprocal+multiply normalize, psout, bf16 output DMA.
"""

from contextlib import ExitStack

import numpy as np
import ml_dtypes

import concourse.bass as bass
import concourse.tile as tile
from concourse import bacc, mybir
from concourse.bass_utils import run_bass_kernel_spmd

BF16 = mybir.dt.bfloat16
F32 = mybir.dt.float32
FP8 = mybir.dt.float8e4
AF = mybir.ActivationFunctionType
AX = mybir.AxisListType
OP = mybir.AluOpType
DR = mybir.MatmulPerfMode.DoubleRow

S = 8192        # sequence length
E = 512         # embedding dim
D = 64          # head dim
L = 64          # landmarks
N_ITER = 6
SCALE = 1.0 / np.sqrt(np.sqrt(D))
W8 = 32.0                    # fp8 / bf16 weight pre-scale for q,k
EXPSC = 1.0 / (W8 * W8)      # removes the 32x q * 32x k factor inside exp

_CACHED_NC = None
DEBUG_TAPS = False


def _build():
    nc = bacc.Bacc("TRN2", target_bir_lowering=False, debug=False, num_devices=8)

    x8_d = nc.dram_tensor("x8", [E, S], FP8, kind="ExternalInput").ap()
    xb_d = nc.dram_tensor("xb", [E, S], BF16, kind="ExternalInput").ap()
    xsegT_d = nc.dram_tensor("xsegT", [E, L], BF16, kind="ExternalInput").ap()
    wqk8_d = nc.dram_tensor("wqk8", [E, 512], FP8, kind="ExternalInput").ap()
    wqkb_d = nc.dram_tensor("wqkb", [E, 512], BF16, kind="ExternalInput").ap()
    wv_d = nc.dram_tensor("wv", [E, 256], BF16, kind="ExternalInput").ap()
    wo_d = nc.dram_tensor("wo", [256, E], BF16, kind="ExternalInput").ap()
    bqk_d = nc.dram_tensor("bqk", [512], F32, kind="ExternalInput").ap()
    idf32_d = nc.dram_tensor("idf32", [128, 128], F32, kind="ExternalInput").ap()
    nsc_d = nc.dram_tensor("nsc", [128, 384], F32, kind="ExternalInput").ap()
    onesr_d = nc.dram_tensor("onesr", [1, 128], F32, kind="ExternalInput").ap()
    blk1_d = nc.dram_tensor("blk1", [128, 128], BF16, kind="ExternalInput").ap()
    out_d = nc.dram_tensor("out", [S, E], BF16, kind="ExternalOutput").ap()
    taps = None
    if DEBUG_TAPS:
        taps = {
            "dbg_landqk": nc.dram_tensor("dbg_landqk", [128, 4, L], BF16,
                                         kind="ExternalOutput").ap(),
            "dbg_qT": nc.dram_tensor("dbg_qT", [128, 2, S], BF16,
                                     kind="ExternalOutput").ap(),
            "dbg_kT": nc.dram_tensor("dbg_kT", [128, 2, S], BF16,
                                     kind="ExternalOutput").ap(),
            "dbg_vsb": nc.dram_tensor("dbg_vsb", [128, 64, 4, 65], BF16,
                                      kind="ExternalOutput").ap(),
            "dbg_e1": nc.dram_tensor("dbg_e1", [128, 2, S], BF16,
                                     kind="ExternalOutput").ap(),
            "dbg_t2": nc.dram_tensor("dbg_t2", [128, 2, 128], BF16,
                                     kind="ExternalOutput").ap(),
        }

    with tile.TileContext(nc) as tc:
        _emit(nc, tc, x8_d, xb_d, xsegT_d, wqk8_d, wqkb_d, wv_d, wo_d, bqk_d,
              idf32_d, nsc_d, onesr_d, blk1_d, out_d, taps)
    nc.compile()
    return nc


def _emit(nc, tc, x8_d, xb_d, xsegT_d, wqk8_d, wqkb_d, wv_d, wo_d, bqk_d,
          idf32_d, nsc_d, onesr_d, blk1_d, out_d, taps=None):
    with (
        tc.tile_pool(name="const", bufs=1) as const,
        tc.tile_pool(name="big", bufs=1) as big,
        tc.tile_pool(name="small", bufs=2) as small,
    ):
        # ---- constants / weights into SBUF ----
        wqkb_sb = const.tile([128, 4, 512], BF16, tag="wqkb")
        nc.sync.dma_start(wqkb_sb[:], wqkb_d.rearrange("(ko p) m -> p ko m", p=128))
        xsegT_sb = const.tile([128, 4, L], BF16, tag="xsegT")
        nc.sync.dma_start(xsegT_sb[:], xsegT_d.rearrange("(ko p) l -> p ko l", p=128))
        bqk_sb = const.tile([128, 4], F32, tag="bqk")
        nc.sync.dma_start(bqk_sb[:], bqk_d.rearrange("(t p) -> p t", p=128))
        wqk8_sb = const.tile([128, 4, 512], FP8, tag="wqk8")
        nc.sync.dma_start(wqk8_sb[:], wqk8_d.rearrange("(ko p) m -> p ko m", p=128))
        wv_sb = const.tile([128, 4, 256], BF16, tag="wv")
        nc.sync.dma_start(wv_sb[:], wv_d.rearrange("(ko p) m -> p ko m", p=128))
        # non-critical consts DMA'd after the first x chunks (see below)
        wo_sb = const.tile([128, 2, 512], BF16, tag="wo")
        idf32_sb = const.tile([128, 128], F32, tag="idf32")
        nsc_sb = const.tile([128, 384], F32, tag="nsc")
        onesr_sb = const.tile([1, 128], F32, tag="onesr")
        blk1_sb = const.tile([128, 128], BF16, tag="blk1")

        # ---- persistent activations ----
        qT = big.tile([128, 2, S], BF16, tag="qT")       # 32x q, [(2h d) | s]
        kT = big.tile([128, 2, S], BF16, tag="kT")
        vsb = big.tile([128, 64, 4, 65], BF16, tag="v")  # [s | chunk, head, d+1]
        e1sb = big.tile([128, 2, S], BF16, tag="e1")     # exp(kernel_1 logits)
        nc.vector.memset(vsb[:, :, :, 64:65], 1.0)

        landqk = const.tile([128, 4, L], BF16, tag="landqk")  # 32x landmarks
        qblk = []
        kblk = []
        t2blk = []
        for hp in range(2):
            qb = const.tile([128, 128], BF16, tag=f"qblk{hp}")
            kb = const.tile([128, 128], BF16, tag=f"kblk{hp}")
            tb = const.tile([128, 128], BF16, tag=f"t2blk{hp}")
            for b_ in (qb, kb, tb):
                nc.vector.memset(b_[:], 0.0)
            qblk.append(qb)
            kblk.append(kb)
            t2blk.append(tb)

        x8_t = x8_d.rearrange("(ko p) s -> p ko s", p=128)
        xb_t = xb_d.rearrange("(ko p) s -> p ko s", p=128)

        HSL = [slice(0, 64), slice(64, 128)]

        # =================== Phase A + NS ===================
        with (
            tc.tile_pool(name="ps_t1", bufs=1, space="PSUM") as ps_t1,
            tc.tile_pool(name="ps_ns", bufs=1, space="PSUM") as ps_ns,
            tc.tile_pool(name="nsp", bufs=2) as nsp,
        ):
            _stk = ExitStack()
            xpool = _stk.enter_context(tc.tile_pool(name="xt", bufs=3))
            ps_qk = _stk.enter_context(
                tc.tile_pool(name="ps_qk", bufs=1, space="PSUM"))
            ps_v = _stk.enter_context(
                tc.tile_pool(name="ps_v", bufs=2, space="PSUM"))
            ps_s3 = _stk.enter_context(
                tc.tile_pool(name="ps_s3", bufs=2, space="PSUM"))
            e3p = _stk.enter_context(tc.tile_pool(name="e3p", bufs=3))
            # prefetch first x chunks ahead of the non-critical consts
            xt8s = {}
            xtbs = {}

            def fetch(c):
                slc = bass.ts(c, 512)
                a = xpool.tile([128, 4, 512], FP8, tag="xt8", name=f"xt8_{c}")
                nc.sync.dma_start(a[:], x8_t[:, :, slc])
                bb = xpool.tile([128, 4, 512], BF16, tag="xtb", name=f"xtb_{c}")
                nc.sync.dma_start(bb[:], xb_t[:, :, slc])
                xt8s[c] = a
                xtbs[c] = bb

            fetch(0)
            fetch(1)
            nc.sync.dma_start(idf32_sb[:], idf32_d[:])
            nc.sync.dma_start(nsc_sb[:], nsc_d[:])
            nc.sync.dma_start(onesr_sb[:], onesr_d[:])
            nc.sync.dma_start(wo_sb[:], wo_d.rearrange("(j p) m -> p j m", p=128))
            nc.sync.dma_start(blk1_sb[:], blk1_d[:])

            # ---- landmarks from host-precomputed segment means ----
            ps_land = ps_ns.tile([128, 4, L], F32, tag="ns", name="ps_land")
            for t in range(4):
                for ko in range(4):
                    nc.tensor.matmul(
                        ps_land[:, t, :], lhsT=wqkb_sb[:, ko, bass.ts(t, 128)],
                        rhs=xsegT_sb[:, ko, :], start=(ko == 0), stop=(ko == 3))
            for t in range(4):
                nc.scalar.activation(landqk[:, t, :], ps_land[:, t, :],
                                     AF.Identity, bias=bqk_sb[:, t:t + 1])
            for hp in range(2):
                for h2 in range(2):
                    psl = HSL[h2]
                    nc.vector.tensor_copy(qblk[hp][psl, psl], landqk[psl, hp, :])
                    nc.vector.tensor_copy(kblk[hp][psl, psl],
                                          landqk[psl, 2 + hp, :])

            # ---- kernel_2 (block-diag, both heads of hp at once) ----
            K2blk = []
            for hp in range(2):
                ps2 = ps_ns.tile([128, 128], F32, tag="ns", name=f"ps2_{hp}")
                nc.tensor.matmul(ps2[:], lhsT=qblk[hp], rhs=kblk[hp],
                                 start=True, stop=True)
                k2e = nsp.tile([128, 128], F32, tag=f"k2e{hp}", name=f"k2e{hp}")
                nc.vector.memset(k2e[:], 0.0)
                rs = small.tile([128, 1], F32, tag="k2rs")
                for h2 in range(2):
                    psl = HSL[h2]
                    nc.scalar.activation(k2e[psl, psl], ps2[psl, psl], AF.Exp,
                                         scale=EXPSC, accum_out=rs[psl, :])
                ri = small.tile([128, 1], F32, tag="k2ri")
                nc.vector.reciprocal(ri[:], rs[:])
                K2 = nsp.tile([128, 128], F32, tag=f"K2_{hp}", name=f"K2_{hp}")
                nc.vector.tensor_scalar_mul(K2[:], k2e[:], ri[:])
                K2blk.append(K2)

            # ---- Newton-Schulz init (block-diag pairs) ----
            K2Tf = []
            K2Tb = []
            Vb = []
            Wb = []
            for hp in range(2):
                pk = ps_ns.tile([128, 128], F32, tag="ns", name=f"pk{hp}")
                nc.tensor.transpose(pk[:], K2blk[hp][:], idf32_sb[:])
                k2tf = nsp.tile([128, 128], F32, tag=f"K2Tf{hp}", name=f"K2Tf{hp}")
                nc.vector.tensor_copy(k2tf[:], pk[:])
                k2tb = nsp.tile([128, 128], BF16, tag=f"K2Tb{hp}", name=f"K2Tb{hp}")
                nc.scalar.activation(k2tb[:], pk[:], AF.Identity)
                K2Tf.append(k2tf)
                K2Tb.append(k2tb)

                cs = nsp.tile([128, 1], F32, tag=f"cs{hp}", name=f"cs{hp}")
                nc.vector.reduce_sum(cs[:], k2tf[:], axis=AX.X)
                pc = ps_ns.tile([1, 128], F32, tag="ns", name=f"pc{hp}")
                nc.tensor.transpose(pc[0:1, :], cs[:], idf32_sb[:])
                # one init scale per head pair (NS convergence is insensitive
                # to a constant factor in the init scale)
                mx = nsp.tile([1, 1], F32, tag=f"mx{hp}", name=f"mx{hp}")
                nc.vector.reduce_max(mx[:], pc[0:1, :], axis=AX.X)
                pb = ps_ns.tile([128, 1], F32, tag="ns", name=f"pb{hp}")
                nc.tensor.matmul(pb[:, 0:1], lhsT=onesr_sb[0:1, :],
                                 rhs=mx[:], start=True, stop=True)
                mxi = nsp.tile([128, 1], F32, tag=f"mxi{hp}", name=f"mxi{hp}")
                nc.vector.reciprocal(mxi[:], pb[:])
                v0 = nsp.tile([128, 128], BF16, tag=f"V{hp}", name=f"V{hp}")
                nc.vector.tensor_scalar_mul(v0[:], k2tf[:], mxi[:])
                w0 = nsp.tile([128, 128], BF16, tag=f"W{hp}", name=f"W{hp}")
                nc.vector.tensor_scalar_mul(w0[:], K2blk[hp][:], mxi[:])
                Vb.append(v0)
                Wb.append(w0)

            # ---- Newton-Schulz iterations ----
            # iters 0..4 in bf16, final iteration in f32
            Vcur, Wcur = Vb, Wb
            for it in range(N_ITER):
                last = it == N_ITER - 1
                dt_i = F32 if last else BF16
                K2T = K2Tf if last else K2Tb
                if last:
                    Vf, Wf = [], []
                    for hp in range(2):
                        vf = nsp.tile([128, 128], F32, tag=f"Vf{hp}",
                                      name=f"Vf{hp}")
                        nc.gpsimd.tensor_copy(vf[:], Vcur[hp][:])
                        wf = nsp.tile([128, 128], F32, tag=f"Wf{hp}",
                                      name=f"Wf{hp}")
                        nc.gpsimd.tensor_copy(wf[:], Wcur[hp][:])
                        Vf.append(vf)
                        Wf.append(wf)
                    Vcur, Wcur = Vf, Wf
                Vnew, Wnew = [], []
                for hp in range(2):
                    pkv = ps_ns.tile([128, 128], F32, tag="ns",
                                     name=f"pkv{it}{hp}")
                    nc.tensor.matmul(pkv[:], lhsT=K2T[hp][:], rhs=Vcur[hp][:],
                                     start=True, stop=True)
                    T1 = nsp.tile([128, 128], dt_i, tag=f"T1{hp}",
                                  name=f"T1_{it}{hp}")
                    nc.vector.tensor_tensor(T1[:], nsc_sb[:, 0:128], pkv[:],
                                            op=OP.subtract)
                    pvt = ps_ns.tile([128, 128], F32, tag="ns",
                                     name=f"pvt{it}{hp}")
                    nc.tensor.matmul(pvt[:], lhsT=Vcur[hp][:], rhs=K2T[hp][:],
                                     start=True, stop=True)
                    KVT = nsp.tile([128, 128], dt_i, tag=f"KVT{hp}",
                                   name=f"KVT_{it}{hp}")
                    nc.scalar.activation(KVT[:], pvt[:], AF.Identity)
                    p3 = ps_ns.tile([128, 128], F32, tag="ns",
                                    name=f"p3{it}{hp}")
                    nc.tensor.matmul(p3[:], lhsT=KVT[:], rhs=T1[:],
                                     start=True, stop=True)
                    T2 = nsp.tile([128, 128], dt_i, tag=f"T2{hp}",
                                  name=f"T2_{it}{hp}")
                    nc.vector.tensor_tensor(T2[:], nsc_sb[:, 128:256], p3[:],
                                            op=OP.subtract)
                    p4 = ps_ns.tile([128, 128], F32, tag="ns",
                                    name=f"p4{it}{hp}")
                    nc.tensor.matmul(p4[:], lhsT=KVT[:], rhs=T2[:],
                                     start=True, stop=True)
                    T3 = nsp.tile([128, 128], dt_i, tag=f"T3{hp}",
                                  name=f"T3_{it}{hp}")
                    nc.vector.scalar_tensor_tensor(
                        T3[:], p4[:], -0.25, nsc_sb[:, 256:384],
                        op0=OP.mult, op1=OP.add)
                    p5 = ps_ns.tile([128, 128], F32, tag="ns",
                                    name=f"p5{it}{hp}")
                    nc.tensor.matmul(p5[:], lhsT=Wcur[hp][:], rhs=T3[:],
                                     start=True, stop=True)
                    p6 = ps_ns.tile([128, 128], F32, tag="ns",
                                    name=f"p6{it}{hp}")
                    nc.tensor.matmul(p6[:], lhsT=T3[:], rhs=Wcur[hp][:],
                                     start=True, stop=True)
                    vn = nsp.tile([128, 128], dt_i, tag=f"V{hp}" if not last
                                  else f"Vf{hp}", name=f"Vn_{it}{hp}")
                    nc.scalar.activation(vn[:], p5[:], AF.Identity)
                    wn = nsp.tile([128, 128], dt_i, tag=f"W{hp}" if not last
                                  else f"Wf{hp}", name=f"Wn_{it}{hp}")
                    nc.scalar.activation(wn[:], p6[:], AF.Identity)
                    Vnew.append(vn)
                    Wnew.append(wn)
                Vcur, Wcur = Vnew, Wnew

            # ---- main chunk loop: qkv projections, e3, t1 accumulation ----
            t1all = ps_t1.tile([65, 256], F32, tag="t1", name="t1ps")
            t1ps = [t1all[:, 0:128], t1all[:, 128:256]]
            # all four (hp, h2) groups share one PSUM bank: a start=True would
            # mark the whole 2KB zero-region and wipe sibling partials, so
            # zero once and accumulate with start=False
            nc.vector.memset(t1all[:], 0.0)
            for c in range(16):
                sl = bass.ts(c, 512)
                if c + 2 < 16:
                    fetch(c + 2)
                xt8 = xt8s.pop(c)
                xtb = xtbs.pop(c)
                for tp in range(2):  # t-pairs: (q01,q23), (k01,k23)
                    ps = ps_qk.tile([128, 2, 512], F32, tag="psqk")
                    for t2 in range(2):
                        t = tp * 2 + t2
                        for kp in range(2):
                            nc.tensor.matmul(
                                ps[:, t2, :],
                                lhsT=wqk8_sb[:, 2 * kp:2 * kp + 2,
                                             bass.ts(t, 128)],
                                rhs=xt8[:, 2 * kp:2 * kp + 2, :],
                                start=(kp == 0), stop=(kp == 1), perf_mode=DR)
                    dst = qT if tp == 0 else kT
                    nc.vector.tensor_scalar_add(
                        dst[:, :, sl], ps[:],
                        bqk_sb[:, 2 * tp:2 * tp + 2].rearrange(
                            "p t -> p t 1"))
                for sp in range(2):  # pairs of 128-row s chunks
                    psv = ps_v.tile([128, 2, 256], F32, tag="psv")
                    for j in range(2):
                        s4 = sp * 2 + j
                        for ko in range(4):
                            nc.tensor.matmul(
                                psv[:, j, :], lhsT=xtb[:, ko, bass.ts(s4, 128)],
                                rhs=wv_sb[:, ko, :], start=(ko == 0),
                                stop=(ko == 3))
                    if (c * 2 + sp) % 3 == 0:
                        nc.vector.tensor_copy(
                            vsb[:, c * 4 + sp * 2:c * 4 + sp * 2 + 2, :, 0:64],
                            psv[:].rearrange("p j (h d) -> p j h d", d=64))
                    else:
                        nc.scalar.activation(
                            vsb[:, c * 4 + sp * 2:c * 4 + sp * 2 + 2, :, 0:64],
                            psv[:].rearrange("p j (h d) -> p j h d", d=64),
                            AF.Identity)
                for hp in range(2):
                    ps3 = ps_s3.tile([128, 512], F32, tag="ps3")
                    for i in range(4):
                        nc.tensor.matmul(
                            ps3[:, bass.ts(i, 128)],
                            lhsT=kT[:, hp, bass.ts(c * 4 + i, 128)],
                            rhs=qblk[hp], start=True, stop=True)
                    e3 = e3p.tile([128, 512], BF16, tag="e3")
                    nc.scalar.activation(e3[:], ps3[:], AF.Exp, scale=EXPSC)
                    for i in range(4):
                        cc = c * 4 + i
                        for h2 in range(2):
                            h = hp * 2 + h2
                            nc.tensor.matmul(
                                t1ps[hp][:, bass.ts(h2, 64)],
                                lhsT=vsb[:, cc, h, :],
                                rhs=e3[:, i * 128 + h2 * 64:
                                       i * 128 + h2 * 64 + 64],
                                start=False, stop=(cc == 63),
                                skip_group_check=True)
                    ps1 = ps_s3.tile([128, 512], F32, tag="ps3", name=f"ps1_{c}{hp}")
                    nc.tensor.matmul(ps1[:], lhsT=kblk[hp], rhs=qT[:, hp, sl],
                                     start=True, stop=True)
                    nc.scalar.activation(e1sb[:, hp, sl], ps1[:], AF.Exp,
                                         scale=EXPSC)

            _stk.close()  # free chunk-loop pools (SBUF + 6 PSUM banks)

            # ---- NS tail: normalize t1, add v bias, t2 = V2 @ t1n ----
            with tc.tile_pool(name="ps_tail", bufs=4, space="PSUM") as ps_tail:
                for hp in range(2):
                    t1u = nsp.tile([65, 128], F32, tag=f"t1u{hp}",
                                   name=f"t1u{hp}")
                    nc.vector.tensor_copy(t1u[:], t1ps[hp][:])
                    ptt = ps_tail.tile([128, 65], F32, tag="tl",
                                       name=f"ptt{hp}")
                    nc.tensor.transpose(ptt[:], t1u[:], idf32_sb[0:65, 0:65])
                    d3i = nsp.tile([128, 1], F32, tag=f"d3i{hp}",
                                   name=f"d3i{hp}")
                    nc.vector.reciprocal(d3i[:], ptt[:, 64:65])
                    # ptt rows are already (h2, l); t1n blocks are [l, d]
                    t1n = nsp.tile([128, 64], F32, tag=f"t1n{hp}",
                                   name=f"t1n{hp}")
                    nc.vector.tensor_scalar_mul(t1n[:], ptt[:, 0:64], d3i[:])
                    t1nblk = nsp.tile([128, 128], F32, tag=f"t1nblk{hp}",
                                      name=f"t1nblk{hp}")
                    nc.vector.memset(t1nblk[:], 0.0)
                    for h2 in range(2):
                        psl = HSL[h2]
                        nc.vector.tensor_copy(t1nblk[psl, psl], t1n[psl, :])
                    pt2 = ps_tail.tile([128, 128], F32, tag="tl",
                                       name=f"pt2{hp}")
                    nc.tensor.matmul(pt2[:], lhsT=Wcur[hp][:], rhs=t1nblk[:],
                                     start=True, stop=True)
                    nc.vector.tensor_copy(t2blk[hp][:], pt2[:])

        # ======= Phase B: kernel_1, normalize, output projection =======
        with (
            tc.tile_pool(name="ps_rb", bufs=2, space="PSUM") as ps_rb,
            tc.tile_pool(name="ps_ht", bufs=2, space="PSUM") as ps_ht,
            tc.tile_pool(name="ps_out", bufs=2, space="PSUM") as ps_out,
            tc.tile_pool(name="hcp", bufs=4) as hcp,
        ):
            for c in range(16):
                sl = bass.ts(c, 512)
                hcts = []
                for hp in range(2):
                    prb = ps_rb.tile([128, 512], F32, tag="prb")
                    nc.tensor.matmul(prb[:], lhsT=blk1_sb[:],
                                     rhs=e1sb[:, hp, sl], start=True, stop=True)
                    pht = ps_ht.tile([128, 512], F32, tag="pht")
                    nc.tensor.matmul(pht[:], lhsT=t2blk[hp],
                                     rhs=e1sb[:, hp, sl], start=True, stop=True)
                    rbs = hcp.tile([128, 512], F32, tag="rbs")
                    nc.vector.reciprocal(rbs[:], prb[:])
                    hct = hcp.tile([128, 512], BF16, tag="hct")
                    nc.vector.tensor_tensor(hct[:], pht[:], rbs[:],
                                            op=OP.mult)
                    hcts.append(hct)
                for s2 in range(2):  # pairs of 128-row output chunks
                    pso = ps_out.tile([128, 2, 512], F32, tag="psout")
                    for j in range(2):
                        s4 = s2 * 2 + j
                        for hp in range(2):
                            nc.tensor.matmul(pso[:, j, :],
                                             lhsT=hcts[hp][:, bass.ts(s4, 128)],
                                             rhs=wo_sb[:, hp, :],
                                             start=(hp == 0), stop=(hp == 1))
                    osb = hcp.tile([128, 2, 512], BF16, tag="osb")
                    nc.scalar.activation(osb[:], pso[:], AF.Identity)
                    nc.sync.dma_start(
                        out_d[bass.ts(c * 2 + s2, 256), :].rearrange(
                            "(j p) m -> p j m", p=128), osb[:])
        if taps is not None:
            nc.sync.dma_start(taps["dbg_landqk"][:], landqk[:])
            nc.sync.dma_start(taps["dbg_qT"][:], qT[:])
            nc.sync.dma_start(taps["dbg_kT"][:], kT[:])
            nc.sync.dma_start(taps["dbg_vsb"][:], vsb[:])
            nc.sync.dma_start(taps["dbg_e1"][:], e1sb[:])
            for hp in range(2):
                nc.sync.dma_start(taps["dbg_t2"][:, hp, :], t2blk[hp][:])


def _prep_inputs(x, Wq, bq, Wk, bk, Wv, bv, Wo, bo):
    bf = ml_dtypes.bfloat16
    f8 = ml_dtypes.float8_e4m3
    x = np.asarray(x, dtype=np.float32)
    Wq = np.asarray(Wq, dtype=np.float32)
    Wk = np.asarray(Wk, dtype=np.float32)
    Wv = np.asarray(Wv, dtype=np.float32)
    Wo = np.asarray(Wo, dtype=np.float32)
    bq = np.asarray(bq, dtype=np.float32)
    bk = np.asarray(bk, dtype=np.float32)
    bv = np.asarray(bv, dtype=np.float32)

    i64 = np.eye(64, dtype=np.float32)
    i128 = np.eye(128, dtype=np.float32)
    consts = {
        "idf32": i128,
        "nsc": np.ascontiguousarray(np.concatenate(
            [7.0 * i128, 15.0 * i128, 3.25 * i128], axis=1)),
        "onesr": np.ones((1, 128), dtype=np.float32),
        "blk1": np.ascontiguousarray(
            np.kron(np.eye(2), np.ones((64, 64))).astype(bf)),
    }
    in_maps = []
    for core in range(8):
        b, g = core // 2, core % 2
        hsl = slice(g * 256, (g + 1) * 256)
        xT = np.ascontiguousarray(x[b].T)
        xseg = x[b].reshape(L, S // L, E).mean(axis=1)  # [L, E]
        wqk = np.ascontiguousarray(
            np.concatenate([Wq[:, hsl], Wk[:, hsl]], axis=1) * (SCALE * W8))
        in_maps.append({
            "x8": xT.astype(f8),
            "xb": xT.astype(bf),
            "xsegT": np.ascontiguousarray(xseg.T).astype(bf),
            "wqk8": wqk.astype(f8),
            "wqkb": wqk.astype(bf),
            "wv": np.ascontiguousarray(Wv[:, hsl]).astype(bf),
            "wo": np.ascontiguousarray(Wo[hsl, :]).astype(bf),
            "bqk": np.ascontiguousarray(
                np.concatenate([bq[hsl], bk[hsl]]) * (SCALE * W8)
            ).astype(np.float32),
            **consts,
        })
    return in_maps


def run_on_device(in_maps, **kwargs):
    global _CACHED_NC
    if _CACHED_NC is None:
        _CACHED_NC = _build()
    return run_bass_kernel_spmd(_CACHED_NC, in_maps, core_ids=list(range(8)),
                                **kwargs)


def kernel(x, Wq, bq, Wk, bk, Wv, bv, Wo, bo):
    in_maps = _prep_inputs(x, Wq, bq, Wk, bk, Wv, bv, Wo, bo)
    res = run_on_device(in_maps)
    bo = np.asarray(bo, dtype=np.float32) + (
        np.asarray(bv, dtype=np.float32) @ np.asarray(Wo, dtype=np.float32))
    out = np.empty((4, S, E), dtype=np.float32)
    for b in range(4):
        out[b] = (res.results[2 * b]["out"].astype(np.float32)
                  + res.results[2 * b + 1]["out"].astype(np.float32) + bo)
    return out
